# revision 7
# baseline (speedup 1.0000x reference)
"""Trainium2 Bass kernel for nn_MultiHeadSelfAttention_49160195670596.

Strategy: tensor-parallel over the 8 heads (one head per NeuronCore).
The reference's torch-style .view from (H*B, L, D) to (B, L, H*D) maps
output batch b' to exactly one head h = b'//2, so each core computes its
two output batches fully locally -- no collectives.

Per core (head h), per batch b:
  q_T[dh,l] = WqT_h.T @ x_T          (+bq_h on the PSUM->SBUF evac;
                                      1/sqrt(D) folded into WqT_h, bq_h)
  k_T[dh,l] = WkT_h.T @ x_T          (+bk_h on evac)
  v[l,dh]   = x_T.T @ WvT_h + bv_h   (bias via K=1 ones-matmul preload)
  s_T[k,q]  = k_T.T @ q_T            (scores transposed: softmax axis=q
                                      becomes the free axis; |s|<~2 so no
                                      max-subtraction is needed)
  e_raw     = exp(s_T)               (ScalarE, straight from PSUM)
  e         = e_raw * keep_T, S[k]=row-sum   (one scalar_tensor_tensor w/
                                      accum_out; keep=!pad_mask staged
                                      transposed in bf16 on host; masked
                                      entries end up exactly 0, matching
                                      the reference's exp(-1e9/sqrt(D)))
  v' = v * (1/S[k])                  (normalizer folded into v)
  att_T[d,q] += v'_i.T @ e_i         (accumulate over 4 k-tiles)
Final projection reads att_T through the torch-view scramble as a strided
AP and produces out_T[d', m]; host transposes/concatenates.

Matmuls run in float32r (single-pass fp32, ~1.5e-4 rel err, 4x faster
than fp32 for N>=256). Softmax-chain work is spread across ScalarE
(exp), VectorE (evacs, reciprocal, v-scale) and GpSimd (mask-multiplies)
to keep all engines below the PE roofline.
"""
import math
import numpy as np
import ml_dtypes

import concourse.bass as bass
import concourse.tile as tile
from concourse import bacc, mybir
from concourse.bass import ts
from concourse.bass_utils import run_bass_kernel_spmd

B, L, D, H = 16, 512, 128, 8
NCORES = 8
KT = L // 128  # 4 k-tiles per batch

f32 = mybir.dt.float32
f32r = mybir.dt.float32r
bf16 = mybir.dt.bfloat16

_CACHE = {}


def _build():
    nc = bacc.Bacc()
    xT_d = nc.dram_tensor("xT", [B, D, L], f32r, kind="ExternalInput")
    mk_d = nc.dram_tensor("keepT", [B, L, L], bf16, kind="ExternalInput")
    wq_d = nc.dram_tensor("wqT", [D, D], f32r, kind="ExternalInput")
    wk_d = nc.dram_tensor("wkT", [D, D], f32r, kind="ExternalInput")
    wv_d = nc.dram_tensor("wvT", [D, D], f32r, kind="ExternalInput")
    bq_d = nc.dram_tensor("bqc", [D, 1], f32, kind="ExternalInput")
    bk_d = nc.dram_tensor("bkc", [D, 1], f32, kind="ExternalInput")
    bv_d = nc.dram_tensor("bvr", [1, L], f32r, kind="ExternalInput")  # bv tiled 4x
    wo_d = nc.dram_tensor("woT", [H * D, D], f32r, kind="ExternalInput")
    bo_d = nc.dram_tensor("bo", [D, 1], f32, kind="ExternalInput")
    on_d = nc.dram_tensor("ones", [1, D], f32r, kind="ExternalInput")
    out_d = nc.dram_tensor("out", [D, 2 * L], f32, kind="ExternalOutput")

    with tile.TileContext(nc) as tc:
        with (
            tc.tile_pool(name="const", bufs=1) as const,
            tc.tile_pool(name="xs", bufs=3) as xs,
            tc.tile_pool(name="mks", bufs=6) as mks,
            tc.tile_pool(name="qks", bufs=2) as qks,
            tc.tile_pool(name="ers", bufs=4) as ers,
            tc.tile_pool(name="es", bufs=6) as es,
            tc.tile_pool(name="vps", bufs=8) as vps,
            tc.tile_pool(name="sts", bufs=4) as sts,
            tc.tile_pool(name="attst", bufs=1) as attst,
            tc.tile_pool(name="outs", bufs=2) as outs,
            tc.tile_pool(name="ps_qk", bufs=2, space="PSUM") as ps_qk,
            tc.tile_pool(name="ps_v", bufs=1, space="PSUM") as ps_v,
            tc.tile_pool(name="ps_sc", bufs=2, space="PSUM") as ps_sc,
            tc.tile_pool(name="ps_at", bufs=2, space="PSUM") as ps_at,
        ):
            wq = const.tile([D, D], f32r)
            nc.sync.dma_start(wq, wq_d[:, :])
            wk = const.tile([D, D], f32r)
            nc.sync.dma_start(wk, wk_d[:, :])
            wv = const.tile([D, D], f32r)
            nc.sync.dma_start(wv, wv_d[:, :])
            bq = const.tile([D, 1], f32)
            nc.sync.dma_start(bq, bq_d[:, :])
            bk = const.tile([D, 1], f32)
            nc.sync.dma_start(bk, bk_d[:, :])
            bv = const.tile([1, L], f32r)
            nc.sync.dma_start(bv, bv_d[:, :])
            bo = const.tile([D, 1], f32)
            nc.sync.dma_start(bo, bo_d[:, :])
            # woT [1024,128] -> SBUF [e=128, j=8, d'=128]
            wo = const.tile([D, H, D], f32r)
            nc.sync.dma_start(wo, wo_d[:, :].rearrange("(j e) d -> e j d", j=H))
            ones = const.tile([1, D], f32r)
            nc.sync.dma_start(ones, on_d[:, :])

            att_store = attst.tile([D, B * L], f32r)

            for b in range(B):
                xT = xs.tile([D, L], f32r)
                nc.sync.dma_start(xT, xT_d[b, :, :])

                # q_T / k_T projections; bias fused into the evacuation
                q_ps = ps_qk.tile([D, L], f32, tag="qk")
                nc.tensor.matmul(q_ps, wq, xT, start=True, stop=True)
                qT = qks.tile([D, L], f32r, tag="q")
                nc.scalar.activation(
                    qT, q_ps, mybir.ActivationFunctionType.Identity, bias=bq
                )

                k_ps = ps_qk.tile([D, L], f32, tag="qk")
                nc.tensor.matmul(k_ps, wk, xT, start=True, stop=True)
                kT = qks.tile([D, L], f32r, tag="k")
                nc.scalar.activation(
                    kT, k_ps, mybir.ActivationFunctionType.Identity, bias=bk
                )

                # v natural [l, dh]: bias preload over the whole bank, then
                # 4 per-l-tile matmuls into its quadrants
                v_ps = ps_v.tile([128, KT, D], f32, tag="v")
                nc.tensor.matmul(
                    v_ps.rearrange("p a b -> p (a b)"), ones,
                    bv, start=True, stop=False, skip_group_check=True,
                )
                for i in range(KT):
                    nc.tensor.matmul(
                        v_ps[:, i, :], xT[:, ts(i, 128)], wv,
                        start=False, stop=True, skip_group_check=True,
                    )

                at_ps = ps_at.tile([D, L], f32, tag="att")
                S = sts.tile([128, KT], f32, tag="S")
                r = sts.tile([128, KT], f32, tag="r")
                e_tiles = []
                for i in range(KT):
                    sc_ps = ps_sc.tile([128, L], f32, tag="sc")
                    nc.tensor.matmul(sc_ps, kT[:, ts(i, 128)], qT, start=True, stop=True)
                    # e_raw = exp(scores) straight from PSUM (no masking yet)
                    er = ers.tile([128, L], f32)
                    nc.scalar.activation(er, sc_ps, mybir.ActivationFunctionType.Exp)
                    # e = e_raw * keep, with fused row-sum -> S[:, i]
                    mk = mks.tile([128, L], bf16)
                    nc.sync.dma_start(mk, mk_d[b, ts(i, 128), :])
                    e = es.tile([128, L], f32r)
                    eng = nc.vector
                    eng.scalar_tensor_tensor(
                        out=e, in0=er, scalar=1.0, in1=mk,
                        op0=mybir.AluOpType.bypass, op1=mybir.AluOpType.mult,
                        accum_out=S[:, i : i + 1],
                    )
                    e_tiles.append(e)
                nc.vector.reciprocal(r, S)
                for i in range(KT):
                    vp = vps.tile([128, D], f32r)
                    nc.vector.tensor_scalar_mul(vp, v_ps[:, i, :], r[:, i : i + 1])
                    nc.tensor.matmul(
                        at_ps, vp, e_tiles[i], start=(i == 0), stop=(i == KT - 1)
                    )

                nc.vector.tensor_copy(att_store[:, ts(b, L)], at_ps)

            # final projection through the torch-view scramble:
            # out_T[d', m] = sum_j woT_j.T @ att_store[:, 4096*half + 8*m + j]
            R = att_store.rearrange("p (h m j) -> p h m j", h=2, j=H)
            for half in range(2):
                o_ps = ps_sc.tile([D, L], f32, tag="sc")
                for j in range(H):
                    nc.tensor.matmul(
                        o_ps, wo[:, j, :], R[:, half, :, j],
                        start=(j == 0), stop=(j == H - 1),
                    )
                ob = outs.tile([D, L], f32)
                nc.vector.tensor_scalar_add(ob, o_ps, bo)
                nc.sync.dma_start(out_d[:, ts(half, L)], ob)

    nc.compile()
    return nc


def _get_nc():
    if "nc" not in _CACHE:
        _CACHE["nc"] = _build()
    return _CACHE["nc"]


def make_in_maps(x, W_q, b_q, W_k, b_k, W_v, b_v, W_o, b_o, pad_mask):
    scale = np.float32(1.0 / math.sqrt(D))
    xT = np.ascontiguousarray(x.transpose(0, 2, 1))  # [B, D, L]
    keepT = np.ascontiguousarray(
        (~pad_mask.transpose(0, 2, 1)).astype(ml_dtypes.bfloat16)
    )  # [B, L(k), L(q)], 1.0 where kept
    woT = np.ascontiguousarray(W_o.T)  # [1024, 128]
    bo_col = np.ascontiguousarray(b_o[:, None])  # [128, 1]
    ones = np.ones((1, D), dtype=np.float32)

    in_maps = []
    for h in range(NCORES):
        sl = slice(h * D, (h + 1) * D)
        in_maps.append(
            {
                "xT": xT,
                "keepT": keepT,
                "wqT": np.ascontiguousarray((W_q[sl, :] * scale).T),
                "wkT": np.ascontiguousarray(W_k[sl, :].T),
                "wvT": np.ascontiguousarray(W_v[sl, :].T),
                "bqc": np.ascontiguousarray((b_q[sl] * scale)[:, None]),
                "bkc": np.ascontiguousarray(b_k[sl][:, None]),
                "bvr": np.ascontiguousarray(np.tile(b_v[sl], KT)[None, :]),
                "woT": woT,
                "bo": bo_col,
                "ones": ones,
            }
        )
    return in_maps


def kernel(x, W_q, b_q, W_k, b_k, W_v, b_v, W_o, b_o, pad_mask, **kwargs):
    x = np.asarray(x, dtype=np.float32)
    W_q = np.asarray(W_q, dtype=np.float32)
    W_k = np.asarray(W_k, dtype=np.float32)
    W_v = np.asarray(W_v, dtype=np.float32)
    W_o = np.asarray(W_o, dtype=np.float32)
    b_q = np.asarray(b_q, dtype=np.float32)
    b_k = np.asarray(b_k, dtype=np.float32)
    b_v = np.asarray(b_v, dtype=np.float32)
    b_o = np.asarray(b_o, dtype=np.float32)
    pad_mask = np.asarray(pad_mask).astype(bool)

    in_maps = make_in_maps(x, W_q, b_q, W_k, b_k, W_v, b_v, W_o, b_o, pad_mask)
    nc = _get_nc()
    res = run_bass_kernel_spmd(nc, in_maps, core_ids=list(range(NCORES)))
    # per-core out_T [128, 1024] -> rows 1024h..1024(h+1) of flat [8192, 128]
    flat = np.concatenate([res.results[h]["out"].T for h in range(NCORES)], axis=0)
    return np.ascontiguousarray(flat.reshape(B, L, D), dtype=np.float32)


if __name__ == "__main__":
    rng = np.random.default_rng(0)
    demo = {
        "x": rng.standard_normal((B, L, D), dtype=np.float32),
        "W_q": rng.standard_normal((H * D, D), dtype=np.float32) * 0.04,
        "b_q": rng.standard_normal(H * D).astype(np.float32) * 0.01,
        "W_k": rng.standard_normal((H * D, D), dtype=np.float32) * 0.04,
        "b_k": rng.standard_normal(H * D).astype(np.float32) * 0.01,
        "W_v": rng.standard_normal((H * D, D), dtype=np.float32) * 0.04,
        "b_v": rng.standard_normal(H * D).astype(np.float32) * 0.01,
        "W_o": rng.standard_normal((D, H * D), dtype=np.float32) * 0.04,
        "b_o": rng.standard_normal(D).astype(np.float32) * 0.01,
        "pad_mask": rng.integers(0, 2, (B, L, L)).astype(bool),
    }
    out = kernel(**demo)
    print("kernel ran, out shape:", out.shape, "finite:", np.isfinite(out).all())
